# revision 49
# baseline (speedup 1.0000x reference)
"""Trainium2 Bass kernel: 12-head self-attention (B=8, N=1024, D=768).

Sharding: data-parallel over batch — one batch element per NeuronCore,
weights replicated on all 8 cores, no collectives.

Per-core dataflow (all matmuls bf16 operands, fp32 PSUM accumulation):
  xT [768,1024] (host-pretransposed, bf16)
  qkT[t] = W_qk[:,t-chunk].T @ xT          (feature-major q/k, 12 chunks)
  v[mt]  = xT[:,mt-chunk].T @ W_v          (token-major v, with a ones
                                            column per head for row sums)
  per head h:
    S^T[mt] = kT_h[:,mt].T @ qT_h          ([keys,queries], K=64 contraction)
    P^T[mt] = exp(scale * S^T[mt])         (ACT, no max-subtraction: scores
                                            are ~N(0,1), exp is safe in f32)
    outT   += v'_h[mt].T @ P^T[mt]         (sums -> PSUM row 0, data rows
                                            64..127)
    attn_T_h = outT[64:128] * bcast(1/outT[0])
  out[nt] = attn_T[:,nt].T @ W_p + b       (bias via broadcast add)

Scheduling: head-sequential software pipeline; 3 rotating PSUM slots for
ST/qkT/v/proj tiles + 1 PV accumulator (8 banks total); PV lags ST/exp by
one mt step; v and future-qkT tiles are interleaved as PE filler inside the
head loops to keep the PE dense (HAM stays at K=8/8).
"""

from contextlib import ExitStack

import numpy as np
import ml_dtypes

import concourse.bacc as bacc
import concourse.bass as bass
import concourse.mybir as mybir
import concourse.tile as tile
from concourse.bass_utils import run_bass_kernel_spmd

B, N, D = 8, 1024, 768
H, HD = 12, 64
SCALE = HD ** -0.5
KC = D // 128          # 6 contraction chunks of 128
NT = N // 128          # 8 token tiles of 128
VW = 128               # per-head v slot: col 0 = ones, cols 64..127 = v data
F32 = mybir.dt.float32
BF16 = mybir.dt.bfloat16
NCORES = 8

_CACHE = {}


def _build_nc():
    nc = bacc.Bacc(None, target_bir_lowering=False)
    xT = nc.dram_tensor("xT", [D, N], BF16, kind="ExternalInput")
    w_qk = nc.dram_tensor("w_qk", [D, 2 * D], BF16, kind="ExternalInput")
    w_v = nc.dram_tensor("w_v", [D, D], BF16, kind="ExternalInput")
    w_p = nc.dram_tensor("w_p", [D, D], BF16, kind="ExternalInput")
    bias = nc.dram_tensor("bias", [1, D], F32, kind="ExternalInput")
    out = nc.dram_tensor("out", [N, D], F32, kind="ExternalOutput")

    with ExitStack() as ctx:
        tc = ctx.enter_context(tile.TileContext(nc))
        const = ctx.enter_context(tc.tile_pool(name="const", bufs=1))
        work = ctx.enter_context(tc.tile_pool(name="work", bufs=2))
        # PSUM: 8 banks total. psA = 3 rotating [128,1024] slots (ST tiles,
        # qkT/v/proj outputs) = 6 banks; psB = 1 slot (PV accumulator) = 2.
        # The 3rd psA slot is what lets ACT run exps back-to-back instead of
        # chaining each exp to the PE's next-ST latency.
        psA = ctx.enter_context(tc.tile_pool(name="psA", bufs=3, space="PSUM"))
        psB = ctx.enter_context(tc.tile_pool(name="psB", bufs=1, space="PSUM"))

        xT_sb = const.tile([128, KC, N], BF16)
        wqk_sb = const.tile([128, KC, 2 * D], BF16)
        wv_sb = const.tile([128, KC, D], BF16)
        wp_sb = const.tile([128, KC, D], BF16)
        bias_sb = const.tile([128, D], F32)
        qk_sb = const.tile([128, 2 * KC, N], BF16)   # chunks 0-5: qT, 6-11: kT
        v_sb = const.tile([128, NT, H * VW], BF16)
        attn_sb = const.tile([128, KC, N], BF16)     # attn_out^T, normalized

        # xT + W_qk per-chunk on the sync (HWDGE) queue so the first qkT
        # matmuls can start early; W_v/W_p/bias on the gpsimd queue.
        for c in range(KC):
            nc.sync.dma_start(out=xT_sb[:, c, :], in_=xT[128 * c:128 * (c + 1), :])
            nc.scalar.dma_start(out=wqk_sb[:, c, :], in_=w_qk[128 * c:128 * (c + 1), :])
        for c in range(KC):
            nc.gpsimd.dma_start(out=wv_sb[:, c, :], in_=w_v[128 * c:128 * (c + 1), :])
            nc.gpsimd.dma_start(out=wp_sb[:, c, :], in_=w_p[128 * c:128 * (c + 1), :])
        bap = bias[:, :]
        bias_bcast = bass.AP(
            tensor=bap.tensor, offset=bap.offset,
            ap=[[0, 128]] + list(bap.ap)[1:],
        )
        nc.gpsimd.dma_start(out=bias_sb, in_=bias_bcast)

        # Per-head v' weights [128 rows of keys, 128 cols]: col 0 = ones
        # (row-sum accumulator -> PSUM partition 0), cols 64..127 = v data
        # (-> PSUM partitions 64..127). Cols 1..63 are zero.
        v4 = v_sb.rearrange("p t (h e) -> p t h e", e=VW)
        nc.gpsimd.memset(v_sb, 0.0)
        nc.gpsimd.memset(v4[:, :, :, 0:1], 1.0)

        def qkT_ops(t):
            """Closures: 6 accumulation-chunk matmul pairs + the cast copy,
            for interleaving as PE filler inside a head's mt loop."""
            ps_qk = psA.tile([128, N], F32, tag="ps", name="ps_qk")
            ops = []
            for c in range(KC):
                def chunk(c=c, ps_qk=ps_qk):
                    for s in range(2):
                        nc.tensor.matmul(
                            ps_qk[:, 512 * s:512 * (s + 1)],
                            lhsT=wqk_sb[:, c, 128 * t:128 * (t + 1)],
                            rhs=xT_sb[:, c, 512 * s:512 * (s + 1)],
                            start=(c == 0), stop=(c == KC - 1),
                        )
                ops.append(chunk)

            def fin(ps_qk=ps_qk):
                nc.vector.tensor_copy(out=qk_sb[:, t, :], in_=ps_qk)
            ops.append(fin)
            return ops

        def emit_qkT(t):
            for op in qkT_ops(t):
                op()

        def emit_v(mt):
            ps_v = psA.tile([128, N], F32, tag="ps", name="ps_v")
            for c in range(KC):
                for lo, sz in ((0, 512), (512, 256)):
                    nc.tensor.matmul(
                        ps_v[:, lo:lo + sz],
                        lhsT=xT_sb[:, c, 128 * mt:128 * (mt + 1)],
                        rhs=wv_sb[:, c, lo:lo + sz],
                        start=(c == 0), stop=(c == KC - 1),
                    )
            nc.vector.tensor_copy(
                out=v4[:, mt, :, 64:128],
                in_=ps_v[:, 0:D].rearrange("p (h e) -> p h e", e=HD),
            )

        ps_o_map = {}

        def emit_ST_exp(h, mt):
            tq, tk = h // 2, KC + h // 2
            po = (h % 2) * 64
            ps_s = psA.tile([128, N], F32, tag="ps", name="ps_s")
            for s in range(2):
                nc.tensor.matmul(
                    ps_s[:, 512 * s:512 * (s + 1)],
                    lhsT=qk_sb[po:po + 64, tk, 128 * mt:128 * (mt + 1)],
                    rhs=qk_sb[po:po + 64, tq, 512 * s:512 * (s + 1)],
                    start=True, stop=True,
                )
            pt = work.tile([128, N], BF16, tag="pt", name="pt", bufs=8)
            nc.scalar.activation(
                out=pt, in_=ps_s,
                func=mybir.ActivationFunctionType.Exp, scale=SCALE,
            )
            return pt

        def emit_PV(h, mt, pt):
            # PV accumulator split into two single-bank [128,512] halves so
            # each half's PSUM slot releases after a half-sized norm chain
            # (halves the slot-release latency blocking the next head's PVs)
            if mt == 0:
                ps_o_map[h] = (
                    psB.tile([128, N // 2], F32, tag="ps", name="ps_oa", bufs=2),
                    psB.tile([128, N // 2], F32, tag="ps", name="ps_ob", bufs=2),
                )
            halves = ps_o_map[h]
            for s in range(2):
                nc.tensor.matmul(
                    halves[s][:, :],
                    lhsT=v_sb[:, mt, VW * h:VW * (h + 1)],
                    rhs=pt[:, 512 * s:512 * (s + 1)],
                    start=(mt == 0), stop=(mt == NT - 1),
                )

        def emit_norm(h):
            # sums on PSUM partition 0; v data on partitions 64..127.
            # (partition_broadcast/reciprocal_approx_fast only read from
            # base partition 0 on HW; DVE ops can't shift partitions.)
            tq = h // 2
            po = (h % 2) * 64
            halves = ps_o_map.pop(h)
            for s in range(2):
                ps_o = halves[s]
                recip = work.tile([1, N // 2], F32, tag="recip", name="recip")
                nc.vector.reciprocal_approx_fast(out=recip, in_=ps_o[0:1, :])
                rb = work.tile([128, N // 2], F32, tag="rb", name="rb")
                nc.gpsimd.partition_broadcast(rb, recip)
                tmp = work.tile([128, N // 2], BF16, tag="tmp", name="tmp")
                nc.vector.tensor_mul(
                    out=tmp[64:128, :], in0=ps_o[64:128, :], in1=rb[64:128, :],
                )
                nc.sync.dma_start(
                    out=attn_sb[po:po + 64, tq, 512 * s:512 * (s + 1)],
                    in_=tmp[64:128, :],
                )

        def emit_proj(nt):
            ps_p = psA.tile([128, N], F32, tag="ps", name="ps_p")
            for c in range(KC):
                for lo, sz in ((0, 512), (512, 256)):
                    nc.tensor.matmul(
                        ps_p[:, lo:lo + sz],
                        lhsT=attn_sb[:, c, 128 * nt:128 * (nt + 1)],
                        rhs=wp_sb[:, c, lo:lo + sz],
                        start=(c == 0), stop=(c == KC - 1),
                    )
            o_sb = work.tile([128, D], F32, tag="o_sb", name="o_sb")
            nc.vector.tensor_add(out=o_sb, in0=ps_p[:, 0:D], in1=bias_sb)
            nc.sync.dma_start(out=out[128 * nt:128 * (nt + 1), :], in_=o_sb)

        emit_qkT(0)
        emit_qkT(KC)
        for mt in range(2):
            emit_v(mt)
        # Filler PE work interleaved inside each head's mt loop: remaining v
        # tiles go into head 0; each pair of heads produces the two qkT tiles
        # needed by the pair two heads later.
        head_fillers = {h: [] for h in range(H)}
        head_fillers[0] = [
            (lambda mt=mt: emit_v(mt)) for mt in range(2, NT)
        ] + qkT_ops(1)
        head_fillers[1] = qkT_ops(KC + 1)
        for k in range(1, KC - 1):
            head_fillers[2 * k] = qkT_ops(k + 1)
            head_fillers[2 * k + 1] = qkT_ops(KC + k + 1)
        # Software pipeline: PV lags ST/exp by one mt step, so the next
        # head's first ST/exp precede the previous head's last PV and the
        # exp stream never breaks at head boundaries.
        pending = []
        for h in range(H):
            fl = head_fillers[h]
            fi = 0
            for mt in range(NT):
                pt = emit_ST_exp(h, mt)
                pending.append((h, mt, pt))
                if len(pending) > 1:
                    ph, pmt, ppt = pending.pop(0)
                    emit_PV(ph, pmt, ppt)
                    if pmt == NT - 1:
                        emit_norm(ph)
                while fi < ((mt + 1) * len(fl) + NT - 1) // NT:
                    fl[fi]()
                    fi += 1
        for ph, pmt, ppt in pending:
            emit_PV(ph, pmt, ppt)
            if pmt == NT - 1:
                emit_norm(ph)
        for nt in range(NT):
            emit_proj(nt)

    nc.compile()
    return nc


def _get_nc():
    if "nc" not in _CACHE:
        _CACHE["nc"] = _build_nc()
    return _CACHE["nc"]


def _make_in_maps(x, W_qkv, W_proj, b_proj):
    bf = ml_dtypes.bfloat16
    x = np.asarray(x, dtype=np.float32)
    W_qkv = np.asarray(W_qkv, dtype=np.float32)
    W_proj = np.asarray(W_proj, dtype=np.float32)
    b_proj = np.asarray(b_proj, dtype=np.float32)
    w_qk = np.ascontiguousarray(W_qkv[:, :2 * D]).astype(bf)
    w_v = np.ascontiguousarray(W_qkv[:, 2 * D:]).astype(bf)
    w_p = W_proj.astype(bf)
    bias = b_proj.reshape(1, D)
    return [
        {
            "xT": np.ascontiguousarray(x[b].T).astype(bf),
            "w_qk": w_qk,
            "w_v": w_v,
            "w_p": w_p,
            "bias": bias,
        }
        for b in range(NCORES)
    ]


def run(x, W_qkv, W_proj, b_proj, trace=False):
    nc = _get_nc()
    in_maps = _make_in_maps(x, W_qkv, W_proj, b_proj)
    res = run_bass_kernel_spmd(nc, in_maps, core_ids=list(range(NCORES)), trace=trace)
    out = np.stack([res.results[b]["out"] for b in range(NCORES)], axis=0)
    return out.astype(np.float32), res


def kernel(x, W_qkv, W_proj, b_proj):
    out, _ = run(x, W_qkv, W_proj, b_proj, trace=False)
    return out


# revision 50
# speedup vs baseline: 1.0965x; 1.0965x over previous
"""Trainium2 Bass kernel: 12-head self-attention (B=8, N=1024, D=768).

Sharding: data-parallel over batch — one batch element per NeuronCore,
weights replicated on all 8 cores, no collectives.

Per-core dataflow (all matmuls bf16 operands, fp32 PSUM accumulation):
  xT [768,1024] (host-pretransposed, bf16)
  qkT[t] = W_qk[:,t-chunk].T @ xT          (feature-major q/k, 12 chunks)
  v[mt]  = xT[:,mt-chunk].T @ W_v          (token-major v, with a ones
                                            column per head for row sums)
  per head h:
    S^T[mt] = kT_h[:,mt].T @ qT_h          ([keys,queries], K=64 contraction)
    P^T[mt] = exp(scale * S^T[mt])         (ACT, no max-subtraction: scores
                                            are ~N(0,1), exp is safe in f32)
    outT   += v'_h[mt].T @ P^T[mt]         (sums -> PSUM row 0, data rows
                                            64..127)
    attn_T_h = outT[64:128] * bcast(1/outT[0])
  out[nt] = attn_T[:,nt].T @ W_p + b       (bias via broadcast add)

Scheduling: head-sequential software pipeline; 3 rotating PSUM slots for
ST/qkT/v/proj tiles + 1 PV accumulator (8 banks total); PV lags ST/exp by
one mt step; v and future-qkT tiles are interleaved as PE filler inside the
head loops to keep the PE dense (HAM stays at K=8/8).
"""

from contextlib import ExitStack

import numpy as np
import ml_dtypes

import concourse.bacc as bacc
import concourse.bass as bass
import concourse.mybir as mybir
import concourse.tile as tile
from concourse.bass_utils import run_bass_kernel_spmd

B, N, D = 8, 1024, 768
H, HD = 12, 64
SCALE = HD ** -0.5
KC = D // 128          # 6 contraction chunks of 128
NT = N // 128          # 8 token tiles of 128
VW = 128               # per-head v slot: col 0 = ones, cols 64..127 = v data
F32 = mybir.dt.float32
BF16 = mybir.dt.bfloat16
NCORES = 8

_CACHE = {}


def _build_nc():
    nc = bacc.Bacc(None, target_bir_lowering=False)
    xT = nc.dram_tensor("xT", [D, N], BF16, kind="ExternalInput")
    w_qk = nc.dram_tensor("w_qk", [D, 2 * D], BF16, kind="ExternalInput")
    w_v = nc.dram_tensor("w_v", [D, D], BF16, kind="ExternalInput")
    w_p = nc.dram_tensor("w_p", [D, D], BF16, kind="ExternalInput")
    bias = nc.dram_tensor("bias", [1, D], F32, kind="ExternalInput")
    out = nc.dram_tensor("out", [N, D], F32, kind="ExternalOutput")

    with ExitStack() as ctx:
        tc = ctx.enter_context(tile.TileContext(nc))
        const = ctx.enter_context(tc.tile_pool(name="const", bufs=1))
        work = ctx.enter_context(tc.tile_pool(name="work", bufs=2))
        # PSUM: 8 banks total. psA = 3 rotating [128,1024] slots (ST tiles,
        # qkT/v/proj outputs) = 6 banks; psB = 1 slot (PV accumulator) = 2.
        # The 3rd psA slot is what lets ACT run exps back-to-back instead of
        # chaining each exp to the PE's next-ST latency.
        psA = ctx.enter_context(tc.tile_pool(name="psA", bufs=3, space="PSUM"))
        psB = ctx.enter_context(tc.tile_pool(name="psB", bufs=1, space="PSUM"))

        xT_sb = const.tile([128, KC, N], BF16)
        wqk_sb = const.tile([128, KC, 2 * D], BF16)
        wv_sb = const.tile([128, KC, D], BF16)
        wp_sb = const.tile([128, KC, D], BF16)
        bias_sb = const.tile([128, D], F32)
        qk_sb = const.tile([128, 2 * KC, N], BF16)   # chunks 0-5: qT, 6-11: kT
        v_sb = const.tile([128, NT, H * VW], BF16)
        attn_sb = const.tile([128, KC, N], BF16)     # attn_out^T, normalized

        # xT + W_qk per-chunk on the sync (HWDGE) queue so the first qkT
        # matmuls can start early; W_v/W_p/bias on the gpsimd queue.
        for c in range(KC):
            nc.sync.dma_start(out=xT_sb[:, c, :], in_=xT[128 * c:128 * (c + 1), :])
            nc.scalar.dma_start(out=wqk_sb[:, c, :], in_=w_qk[128 * c:128 * (c + 1), :])
        for c in range(KC):
            nc.gpsimd.dma_start(out=wv_sb[:, c, :], in_=w_v[128 * c:128 * (c + 1), :])
            nc.gpsimd.dma_start(out=wp_sb[:, c, :], in_=w_p[128 * c:128 * (c + 1), :])
        bap = bias[:, :]
        bias_bcast = bass.AP(
            tensor=bap.tensor, offset=bap.offset,
            ap=[[0, 128]] + list(bap.ap)[1:],
        )
        nc.gpsimd.dma_start(out=bias_sb, in_=bias_bcast)

        # Per-head v' weights [128 rows of keys, 128 cols]: col 0 = ones
        # (row-sum accumulator -> PSUM partition 0), cols 64..127 = v data
        # (-> PSUM partitions 64..127). Cols 1..63 are zero.
        v4 = v_sb.rearrange("p t (h e) -> p t h e", e=VW)
        nc.gpsimd.memset(v_sb, 0.0)
        nc.gpsimd.memset(v4[:, :, :, 0:1], 1.0)

        def qkT_ops(t):
            """Closures: 6 accumulation-chunk matmul pairs + the cast copy,
            for interleaving as PE filler inside a head's mt loop."""
            ps_qk = psA.tile([128, N], F32, tag="ps", name="ps_qk")
            ops = []
            for c in range(KC):
                def chunk(c=c, ps_qk=ps_qk):
                    for s in range(2):
                        nc.tensor.matmul(
                            ps_qk[:, 512 * s:512 * (s + 1)],
                            lhsT=wqk_sb[:, c, 128 * t:128 * (t + 1)],
                            rhs=xT_sb[:, c, 512 * s:512 * (s + 1)],
                            start=(c == 0), stop=(c == KC - 1),
                        )
                ops.append(chunk)

            def fin(ps_qk=ps_qk):
                nc.vector.tensor_copy(out=qk_sb[:, t, :], in_=ps_qk)
            ops.append(fin)
            return ops

        def emit_qkT(t):
            for op in qkT_ops(t):
                op()

        def emit_v(mt):
            ps_v = psA.tile([128, N], F32, tag="ps", name="ps_v")
            for c in range(KC):
                for lo, sz in ((0, 512), (512, 256)):
                    nc.tensor.matmul(
                        ps_v[:, lo:lo + sz],
                        lhsT=xT_sb[:, c, 128 * mt:128 * (mt + 1)],
                        rhs=wv_sb[:, c, lo:lo + sz],
                        start=(c == 0), stop=(c == KC - 1),
                    )
            nc.vector.tensor_copy(
                out=v4[:, mt, :, 64:128],
                in_=ps_v[:, 0:D].rearrange("p (h e) -> p h e", e=HD),
            )

        ps_o_map = {}

        def emit_ST_exp(h, mt):
            tq, tk = h // 2, KC + h // 2
            po = (h % 2) * 64
            ps_s = psA.tile([128, N], F32, tag="ps", name="ps_s")
            for s in range(2):
                nc.tensor.matmul(
                    ps_s[:, 512 * s:512 * (s + 1)],
                    lhsT=qk_sb[po:po + 64, tk, 128 * mt:128 * (mt + 1)],
                    rhs=qk_sb[po:po + 64, tq, 512 * s:512 * (s + 1)],
                    start=True, stop=True,
                )
            pt = work.tile([128, N], BF16, tag="pt", name="pt", bufs=8)
            nc.scalar.activation(
                out=pt, in_=ps_s,
                func=mybir.ActivationFunctionType.Exp, scale=SCALE,
            )
            return pt

        def emit_PV(h, mt, pt):
            if mt == 0:
                ps_o_map[h] = psB.tile([128, N], F32, tag="ps", name="ps_o")
            ps_o = ps_o_map[h]
            for s in range(2):
                nc.tensor.matmul(
                    ps_o[:, 512 * s:512 * (s + 1)],
                    lhsT=v_sb[:, mt, VW * h:VW * (h + 1)],
                    rhs=pt[:, 512 * s:512 * (s + 1)],
                    start=(mt == 0), stop=(mt == NT - 1),
                )

        def emit_norm(h):
            # sums on PSUM partition 0; v data on partitions 64..127.
            # (partition_broadcast/reciprocal_approx_fast only read from
            # base partition 0 on HW; DVE ops can't shift partitions.)
            tq = h // 2
            po = (h % 2) * 64
            ps_o = ps_o_map.pop(h)
            recip = work.tile([1, N], F32, tag="recip", name="recip")
            nc.vector.reciprocal_approx_fast(out=recip, in_=ps_o[0:1, :])
            rb = work.tile([128, N], F32, tag="rb", name="rb")
            nc.gpsimd.partition_broadcast(rb, recip)
            tmp = work.tile([128, N], BF16, tag="tmp", name="tmp")
            nc.vector.tensor_mul(
                out=tmp[64:128, :], in0=ps_o[64:128, :], in1=rb[64:128, :],
            )
            nc.sync.dma_start(
                out=attn_sb[po:po + 64, tq, :], in_=tmp[64:128, :],
            )

        def emit_proj(nt):
            ps_p = psA.tile([128, N], F32, tag="ps", name="ps_p")
            for c in range(KC):
                for lo, sz in ((0, 512), (512, 256)):
                    nc.tensor.matmul(
                        ps_p[:, lo:lo + sz],
                        lhsT=attn_sb[:, c, 128 * nt:128 * (nt + 1)],
                        rhs=wp_sb[:, c, lo:lo + sz],
                        start=(c == 0), stop=(c == KC - 1),
                    )
            o_sb = work.tile([128, D], F32, tag="o_sb", name="o_sb")
            nc.vector.tensor_add(out=o_sb, in0=ps_p[:, 0:D], in1=bias_sb)
            nc.sync.dma_start(out=out[128 * nt:128 * (nt + 1), :], in_=o_sb)

        emit_qkT(0)
        emit_qkT(KC)
        for mt in range(2):
            emit_v(mt)
        # Filler PE work interleaved inside each head's mt loop: remaining v
        # tiles go into head 0; each pair of heads produces the two qkT tiles
        # needed by the pair two heads later.
        head_fillers = {h: [] for h in range(H)}
        head_fillers[0] = [
            (lambda mt=mt: emit_v(mt)) for mt in range(2, NT)
        ] + qkT_ops(1)
        head_fillers[1] = qkT_ops(KC + 1)
        for k in range(1, KC - 1):
            head_fillers[2 * k] = qkT_ops(k + 1)
            head_fillers[2 * k + 1] = qkT_ops(KC + k + 1)
        # Software pipeline: PV lags ST/exp by one mt step, so the next
        # head's first ST/exp precede the previous head's last PV and the
        # exp stream never breaks at head boundaries.
        pending = []
        for h in range(H):
            fl = head_fillers[h]
            fi = 0
            for mt in range(NT):
                pt = emit_ST_exp(h, mt)
                pending.append((h, mt, pt))
                if len(pending) > 1:
                    ph, pmt, ppt = pending.pop(0)
                    emit_PV(ph, pmt, ppt)
                    if pmt == NT - 1:
                        emit_norm(ph)
                while fi < ((mt + 1) * len(fl) + NT - 1) // NT:
                    fl[fi]()
                    fi += 1
        for ph, pmt, ppt in pending:
            emit_PV(ph, pmt, ppt)
            if pmt == NT - 1:
                emit_norm(ph)
        for nt in range(NT):
            emit_proj(nt)

    nc.compile()
    return nc


def _get_nc():
    if "nc" not in _CACHE:
        _CACHE["nc"] = _build_nc()
    return _CACHE["nc"]


def _make_in_maps(x, W_qkv, W_proj, b_proj):
    bf = ml_dtypes.bfloat16
    x = np.asarray(x, dtype=np.float32)
    W_qkv = np.asarray(W_qkv, dtype=np.float32)
    W_proj = np.asarray(W_proj, dtype=np.float32)
    b_proj = np.asarray(b_proj, dtype=np.float32)
    w_qk = np.ascontiguousarray(W_qkv[:, :2 * D]).astype(bf)
    w_v = np.ascontiguousarray(W_qkv[:, 2 * D:]).astype(bf)
    w_p = W_proj.astype(bf)
    bias = b_proj.reshape(1, D)
    return [
        {
            "xT": np.ascontiguousarray(x[b].T).astype(bf),
            "w_qk": w_qk,
            "w_v": w_v,
            "w_p": w_p,
            "bias": bias,
        }
        for b in range(NCORES)
    ]


def run(x, W_qkv, W_proj, b_proj, trace=False):
    nc = _get_nc()
    in_maps = _make_in_maps(x, W_qkv, W_proj, b_proj)
    res = run_bass_kernel_spmd(nc, in_maps, core_ids=list(range(NCORES)), trace=trace)
    out = np.stack([res.results[b]["out"] for b in range(NCORES)], axis=0)
    return out.astype(np.float32), res


def kernel(x, W_qkv, W_proj, b_proj):
    out, _ = run(x, W_qkv, W_proj, b_proj, trace=False)
    return out


# revision 53
# speedup vs baseline: 1.1310x; 1.0314x over previous
"""Trainium2 Bass kernel: 12-head self-attention (B=8, N=1024, D=768).

Sharding: data-parallel over batch — one batch element per NeuronCore,
weights replicated on all 8 cores, no collectives.

Per-core dataflow (all matmuls bf16 operands, fp32 PSUM accumulation):
  xT [768,1024] (host-pretransposed, bf16)
  qkT[t] = W_qk[:,t-chunk].T @ xT          (feature-major q/k, 12 chunks)
  v[mt]  = xT[:,mt-chunk].T @ W_v          (token-major v, with a ones
                                            column per head for row sums)
  per head h:
    S^T[mt] = kT_h[:,mt].T @ qT_h          ([keys,queries], K=64 contraction)
    P^T[mt] = exp(scale * S^T[mt])         (ACT, no max-subtraction: scores
                                            are ~N(0,1), exp is safe in f32)
    outT   += v'_h[mt].T @ P^T[mt]         (sums -> PSUM row 0, data rows
                                            64..127)
    attn_T_h = outT[64:128] * bcast(1/outT[0])
  out[nt] = attn_T[:,nt].T @ W_p + b       (bias via broadcast add)

Scheduling: head-sequential software pipeline; 3 rotating PSUM slots for
ST/qkT/v/proj tiles + 1 PV accumulator (8 banks total); PV lags ST/exp by
one mt step; v and future-qkT tiles are interleaved as PE filler inside the
head loops to keep the PE dense (HAM stays at K=8/8).
"""

from contextlib import ExitStack

import numpy as np
import ml_dtypes

import concourse.bacc as bacc
import concourse.bass as bass
import concourse.mybir as mybir
import concourse.tile as tile
from concourse.bass_utils import run_bass_kernel_spmd

B, N, D = 8, 1024, 768
H, HD = 12, 64
SCALE = HD ** -0.5
KC = D // 128          # 6 contraction chunks of 128
NT = N // 128          # 8 token tiles of 128
VW = 128               # per-head v slot: col 0 = ones, cols 64..127 = v data
F32 = mybir.dt.float32
BF16 = mybir.dt.bfloat16
NCORES = 8

_CACHE = {}


def _build_nc():
    nc = bacc.Bacc(None, target_bir_lowering=False)
    xT = nc.dram_tensor("xT", [D, N], BF16, kind="ExternalInput")
    w_qk = nc.dram_tensor("w_qk", [D, 2 * D], BF16, kind="ExternalInput")
    w_v = nc.dram_tensor("w_v", [D, D], BF16, kind="ExternalInput")
    w_p = nc.dram_tensor("w_p", [D, D], BF16, kind="ExternalInput")
    bias = nc.dram_tensor("bias", [1, D], F32, kind="ExternalInput")
    out = nc.dram_tensor("out", [N, D], F32, kind="ExternalOutput")

    with ExitStack() as ctx:
        tc = ctx.enter_context(tile.TileContext(nc))
        const = ctx.enter_context(tc.tile_pool(name="const", bufs=1))
        work = ctx.enter_context(tc.tile_pool(name="work", bufs=2))
        # PSUM: 8 banks total. psA = 3 rotating [128,1024] slots (ST tiles,
        # qkT/v/proj outputs) = 6 banks; psB = 1 slot (PV accumulator) = 2.
        # The 3rd psA slot is what lets ACT run exps back-to-back instead of
        # chaining each exp to the PE's next-ST latency.
        psA = ctx.enter_context(tc.tile_pool(name="psA", bufs=3, space="PSUM"))
        psB = ctx.enter_context(tc.tile_pool(name="psB", bufs=1, space="PSUM"))

        xT_sb = const.tile([128, KC, N], BF16)
        wqk_sb = const.tile([128, KC, 2 * D], BF16)
        wv_sb = const.tile([128, KC, D], BF16)
        wp_sb = const.tile([128, KC, D], BF16)
        bias_sb = const.tile([128, D], F32)
        qk_sb = const.tile([128, 2 * KC, N], BF16)   # chunks 0-5: qT, 6-11: kT
        v_sb = const.tile([128, NT, H * VW], BF16)
        attn_sb = const.tile([128, KC, N], BF16)     # attn_out^T, normalized

        # xT + W_qk per-chunk on the sync (HWDGE) queue so the first qkT
        # matmuls can start early; W_v/W_p/bias on the gpsimd queue.
        for c in range(KC):
            nc.sync.dma_start(out=xT_sb[:, c, :], in_=xT[128 * c:128 * (c + 1), :])
            nc.scalar.dma_start(out=wqk_sb[:, c, :], in_=w_qk[128 * c:128 * (c + 1), :])
        for c in range(KC):
            nc.gpsimd.dma_start(out=wv_sb[:, c, :], in_=w_v[128 * c:128 * (c + 1), :])
            nc.gpsimd.dma_start(out=wp_sb[:, c, :], in_=w_p[128 * c:128 * (c + 1), :])
        bap = bias[:, :]
        bias_bcast = bass.AP(
            tensor=bap.tensor, offset=bap.offset,
            ap=[[0, 128]] + list(bap.ap)[1:],
        )
        nc.gpsimd.dma_start(out=bias_sb, in_=bias_bcast)

        # Per-head v' weights [128 rows of keys, 128 cols]: col 0 = ones
        # (row-sum accumulator -> PSUM partition 0), cols 64..127 = v data
        # (-> PSUM partitions 64..127). Cols 1..63 are zero.
        v4 = v_sb.rearrange("p t (h e) -> p t h e", e=VW)
        nc.gpsimd.memset(v_sb, 0.0)
        nc.gpsimd.memset(v4[:, :, :, 0:1], 1.0)

        def qkT_ops(t):
            """Closures: 6 accumulation-chunk matmul pairs + the cast copy,
            for interleaving as PE filler inside a head's mt loop."""
            ps_qk = psA.tile([128, N], F32, tag="ps", name="ps_qk")
            ops = []
            for c in range(KC):
                def chunk(c=c, ps_qk=ps_qk):
                    for s in range(2):
                        nc.tensor.matmul(
                            ps_qk[:, 512 * s:512 * (s + 1)],
                            lhsT=wqk_sb[:, c, 128 * t:128 * (t + 1)],
                            rhs=xT_sb[:, c, 512 * s:512 * (s + 1)],
                            start=(c == 0), stop=(c == KC - 1),
                        )
                ops.append(chunk)

            def fin(ps_qk=ps_qk):
                nc.vector.tensor_copy(out=qk_sb[:, t, :], in_=ps_qk)
            ops.append(fin)
            return ops

        def emit_qkT(t):
            for op in qkT_ops(t):
                op()

        def emit_v(mt):
            ps_v = psA.tile([128, N], F32, tag="ps", name="ps_v")
            for c in range(KC):
                for lo, sz in ((0, 512), (512, 256)):
                    nc.tensor.matmul(
                        ps_v[:, lo:lo + sz],
                        lhsT=xT_sb[:, c, 128 * mt:128 * (mt + 1)],
                        rhs=wv_sb[:, c, lo:lo + sz],
                        start=(c == 0), stop=(c == KC - 1),
                    )
            nc.vector.tensor_copy(
                out=v4[:, mt, :, 64:128],
                in_=ps_v[:, 0:D].rearrange("p (h e) -> p h e", e=HD),
            )

        ps_o_map = {}

        def emit_ST_exp(h, mt):
            tq, tk = h // 2, KC + h // 2
            po = (h % 2) * 64
            ps_s = psA.tile([128, N], F32, tag="ps", name="ps_s")
            for s in range(2):
                nc.tensor.matmul(
                    ps_s[:, 512 * s:512 * (s + 1)],
                    lhsT=qk_sb[po:po + 64, tk, 128 * mt:128 * (mt + 1)],
                    rhs=qk_sb[po:po + 64, tq, 512 * s:512 * (s + 1)],
                    start=True, stop=True,
                )
            pt = work.tile([128, N], BF16, tag="pt", name="pt", bufs=8)
            nc.scalar.activation(
                out=pt, in_=ps_s,
                func=mybir.ActivationFunctionType.Exp, scale=SCALE,
            )
            return pt

        def emit_PV(h, mt, pt):
            if mt == 0:
                ps_o_map[h] = psB.tile([128, N], F32, tag="ps", name="ps_o")
            ps_o = ps_o_map[h]
            for s in range(2):
                nc.tensor.matmul(
                    ps_o[:, 512 * s:512 * (s + 1)],
                    lhsT=v_sb[:, mt, VW * h:VW * (h + 1)],
                    rhs=pt[:, 512 * s:512 * (s + 1)],
                    start=(mt == 0), stop=(mt == NT - 1),
                )

        def emit_norm(h):
            # sums on PSUM partition 0; v data on partitions 64..127.
            # (partition_broadcast/reciprocal_approx_fast only read from
            # base partition 0 on HW; DVE ops can't shift partitions.)
            tq = h // 2
            po = (h % 2) * 64
            ps_o = ps_o_map.pop(h)
            recip = work.tile([1, N], F32, tag="recip", name="recip")
            nc.vector.reciprocal_approx_fast(out=recip, in_=ps_o[0:1, :])
            rb = work.tile([128, N], F32, tag="rb", name="rb")
            nc.gpsimd.partition_broadcast(rb, recip)
            if po == 64:
                # odd head: rows already partition-aligned with the attn
                # chunk — multiply straight into attn_sb, no tmp/DMA hop
                nc.vector.tensor_mul(
                    out=attn_sb[64:128, tq, :],
                    in0=ps_o[64:128, :], in1=rb[64:128, :],
                )
            else:
                tmp = work.tile([128, N], BF16, tag="tmp", name="tmp")
                nc.vector.tensor_mul(
                    out=tmp[64:128, :], in0=ps_o[64:128, :], in1=rb[64:128, :],
                )
                nc.sync.dma_start(
                    out=attn_sb[0:64, tq, :], in_=tmp[64:128, :],
                )

        def emit_proj(nt):
            ps_p = psA.tile([128, N], F32, tag="ps", name="ps_p")
            for c in range(KC):
                for lo, sz in ((0, 512), (512, 256)):
                    nc.tensor.matmul(
                        ps_p[:, lo:lo + sz],
                        lhsT=attn_sb[:, c, 128 * nt:128 * (nt + 1)],
                        rhs=wp_sb[:, c, lo:lo + sz],
                        start=(c == 0), stop=(c == KC - 1),
                    )
            o_sb = work.tile([128, D], F32, tag="o_sb", name="o_sb")
            nc.vector.tensor_add(out=o_sb, in0=ps_p[:, 0:D], in1=bias_sb)
            nc.sync.dma_start(out=out[128 * nt:128 * (nt + 1), :], in_=o_sb)

        emit_qkT(0)
        emit_qkT(KC)
        for mt in range(2):
            emit_v(mt)
        # Filler PE work interleaved inside each head's mt loop: remaining v
        # tiles go into head 0; each pair of heads produces the two qkT tiles
        # needed by the pair two heads later.
        head_fillers = {h: [] for h in range(H)}
        head_fillers[0] = [
            (lambda mt=mt: emit_v(mt)) for mt in range(2, NT)
        ] + qkT_ops(1)
        head_fillers[1] = qkT_ops(KC + 1)
        for k in range(1, KC - 1):
            head_fillers[2 * k] = qkT_ops(k + 1)
            head_fillers[2 * k + 1] = qkT_ops(KC + k + 1)
        # Software pipeline: PV lags ST/exp by one mt step, so the next
        # head's first ST/exp precede the previous head's last PV and the
        # exp stream never breaks at head boundaries.
        pending = []
        for h in range(H):
            fl = head_fillers[h]
            fi = 0
            for mt in range(NT):
                pt = emit_ST_exp(h, mt)
                pending.append((h, mt, pt))
                if len(pending) > 1:
                    ph, pmt, ppt = pending.pop(0)
                    emit_PV(ph, pmt, ppt)
                    if pmt == NT - 1:
                        emit_norm(ph)
                while fi < ((mt + 1) * len(fl) + NT - 1) // NT:
                    fl[fi]()
                    fi += 1
        for ph, pmt, ppt in pending:
            emit_PV(ph, pmt, ppt)
            if pmt == NT - 1:
                emit_norm(ph)
        for nt in range(NT):
            emit_proj(nt)

    nc.compile()
    return nc


def _get_nc():
    if "nc" not in _CACHE:
        _CACHE["nc"] = _build_nc()
    return _CACHE["nc"]


def _make_in_maps(x, W_qkv, W_proj, b_proj):
    bf = ml_dtypes.bfloat16
    x = np.asarray(x, dtype=np.float32)
    W_qkv = np.asarray(W_qkv, dtype=np.float32)
    W_proj = np.asarray(W_proj, dtype=np.float32)
    b_proj = np.asarray(b_proj, dtype=np.float32)
    w_qk = np.ascontiguousarray(W_qkv[:, :2 * D]).astype(bf)
    w_v = np.ascontiguousarray(W_qkv[:, 2 * D:]).astype(bf)
    w_p = W_proj.astype(bf)
    bias = b_proj.reshape(1, D)
    return [
        {
            "xT": np.ascontiguousarray(x[b].T).astype(bf),
            "w_qk": w_qk,
            "w_v": w_v,
            "w_p": w_p,
            "bias": bias,
        }
        for b in range(NCORES)
    ]


def run(x, W_qkv, W_proj, b_proj, trace=False):
    nc = _get_nc()
    in_maps = _make_in_maps(x, W_qkv, W_proj, b_proj)
    res = run_bass_kernel_spmd(nc, in_maps, core_ids=list(range(NCORES)), trace=trace)
    out = np.stack([res.results[b]["out"] for b in range(NCORES)], axis=0)
    return out.astype(np.float32), res


def kernel(x, W_qkv, W_proj, b_proj):
    out, _ = run(x, W_qkv, W_proj, b_proj, trace=False)
    return out
